# revision 10
# baseline (speedup 1.0000x reference)
# Multi-head attention layer on 8 TRN2 NeuronCores (SPMD, no collectives).
#
# Problem: B=4, N=2048, D=512, H=8 heads (DK=64).
#   out = softmax((q@Wq+bq)(k@Wk+bk)^T / 8) (v@Wv+bv) @ Wo + bo   per (batch, head)
#
# Sharding: core c handles batch b=c//2 and query-row half c%2 (1024 rows).
# K/V projections are recomputed by both cores of a pair (cheap) so there is
# no cross-core communication at all.
#
# Per-core dataflow (all layouts chosen so NO on-chip transposes are needed;
# the host pre-transposes inputs to (channel, token) layout and casts bf16):
#   K^T(d,k)  = Wk-chunks.T @ kT          (PE), +bias via DVE copy
#   Q^T(d,q)  = Wq-chunks.T @ qT          (PE), +bias via DVE copy
#   V(k,d)    = vT-chunks.T @ Wv          (PE), +bias via Pool add, stored
#               per-head as [V(64)|ones(64)] blocks (pitch 128)
#   S^T(k,q)  = K^T_tile.T @ Q^T  per head  (PE, contraction d=64); the two
#               heads of a pair sit on partitions 0-63 / 64-127 so their
#               matmuls land on disjoint PE row-groups and run CONCURRENTLY
#               (tile_position row tiling, auto-derived from base partitions)
#   P^T       = exp(S^T/8)                (ACT, scale folded into activation)
#   ctx^T(d,q)= [V|1s]-block.T @ P^T      (PE, accumulated over k; the ones
#               BLOCK replicates the softmax denominator across psum rows
#               64-127 -> no cross-partition broadcast needed afterwards)
#   norm      = ctx^T * recip(rowsum)     (DVE reciprocal_approx_fast +
#               SBUF->SBUF DMA partition shift + DVE multiply)
#   out(n,d)  = ctxn-chunks.T @ Wo        (PE), +bias via DVE add of a
#               pre-broadcast bias tile
from contextlib import ExitStack

import numpy as np
import ml_dtypes

import concourse.bass as bass
import concourse.mybir as mybir
import concourse.tile as tile
from concourse import bacc
from concourse.bass_utils import run_bass_kernel_spmd

BF16 = mybir.dt.bfloat16
F32 = mybir.dt.float32
I32 = mybir.dt.int32
Exp = mybir.ActivationFunctionType.Exp

B, N, D, H = 4, 2048, 512, 8
DK = D // H          # 64
NQ = N // 2          # 1024 query rows per core
NKT = N // 128       # 16 k tiles

# Schraudolph fast-exp constants (exp(x/8) ~ bitcast(int32(A*x + BC))).
# Error <= ~3% per attention weight; cancels in the softmax ratio down to
# ~2e-3 on the final output (validated numerically). Used to offload a
# quarter of the exp work from the Activation engine to DVE+Pool.
SCH_A = float(2.0 ** 23 / np.log(2.0) * 0.125)
SCH_B = float(127 * 2 ** 23 - 366393.0)


def build_nc(dbg=False):
    nc = bacc.Bacc("TRN2", target_bir_lowering=False)

    qT = nc.dram_tensor("qT", (D, NQ), BF16, kind="ExternalInput")
    kT = nc.dram_tensor("kT", (D, N), BF16, kind="ExternalInput")
    vT = nc.dram_tensor("vT", (D, N), BF16, kind="ExternalInput")
    wq = nc.dram_tensor("wq", (D, D), BF16, kind="ExternalInput")
    wk = nc.dram_tensor("wk", (D, D), BF16, kind="ExternalInput")
    wv = nc.dram_tensor("wv", (D, D), BF16, kind="ExternalInput")
    wo = nc.dram_tensor("wo", (D, D), BF16, kind="ExternalInput")
    bq = nc.dram_tensor("bq", (D, 1), F32, kind="ExternalInput")
    bk = nc.dram_tensor("bk", (D, 1), F32, kind="ExternalInput")
    bv = nc.dram_tensor("bv", (1, D), BF16, kind="ExternalInput")
    bo = nc.dram_tensor("bo", (1, D), F32, kind="ExternalInput")
    out = nc.dram_tensor("out", (NQ, D), F32, kind="ExternalOutput")

    dbg_t = None
    if dbg:
        dbg_t = {
            "d_kt": nc.dram_tensor("d_kt", (128, 4 * N), BF16, kind="ExternalOutput"),
            "d_qt": nc.dram_tensor("d_qt", (128, 4 * NQ), BF16, kind="ExternalOutput"),
            "d_v": nc.dram_tensor("d_v", (128, NKT * H * 128), BF16, kind="ExternalOutput"),
            "d_ctxe": nc.dram_tensor("d_ctxe", (128, NQ), F32, kind="ExternalOutput"),
            "d_ctxo": nc.dram_tensor("d_ctxo", (128, NQ), F32, kind="ExternalOutput"),
            "d_rece": nc.dram_tensor("d_rece", (128, NQ), F32, kind="ExternalOutput"),
            "d_ctxn": nc.dram_tensor("d_ctxn", (128, 4 * NQ), BF16, kind="ExternalOutput"),
        }

    with tile.TileContext(nc) as tc:
        with ExitStack() as ctx:
            emit(ctx, tc, qT, kT, vT, wq, wk, wv, wo, bq, bk, bv, bo, out, dbg_t)
    nc.compile()
    return nc


def emit(ctx, tc, qT, kT, vT, wq, wk, wv, wo, bq, bk, bv, bo, out, dbg=None):
    nc = tc.nc
    consts = ctx.enter_context(tc.tile_pool(name="consts", bufs=1))
    p_pool = ctx.enter_context(tc.tile_pool(name="p_pool", bufs=6))
    post = ctx.enter_context(tc.tile_pool(name="post", bufs=2))
    outs = ctx.enter_context(tc.tile_pool(name="outs", bufs=3))
    s_pool = ctx.enter_context(tc.tile_pool(name="s_pool", bufs=2, space="PSUM"))
    c_pool = ctx.enter_context(tc.tile_pool(name="c_pool", bufs=2, space="PSUM"))

    # ---- constants / inputs -------------------------------------------------
    def load(name, shape, dt_, src_ap):
        t = consts.tile(shape, dt_, name=name)
        nc.sync.dma_start(out=t, in_=src_ap)
        return t

    # kproj/qproj inputs first so the softmax pipeline starts ASAP
    wk_s = load("wk_s", [128, 4, D], BF16, wk[:].rearrange("(c p) d -> p c d", p=128))
    kT_s = load("kT_s", [128, 4, N], BF16, kT[:].rearrange("(c p) n -> p c n", p=128))
    wq_s = load("wq_s", [128, 4, D], BF16, wq[:].rearrange("(c p) d -> p c d", p=128))
    qT_s = load("qT_s", [128, 4, NQ], BF16, qT[:].rearrange("(c p) n -> p c n", p=128))
    bq_s = load("bq_s", [128, 4, 1], F32, bq[:].rearrange("(c p) o -> p c o", p=128))
    bk_s = load("bk_s", [128, 4, 1], F32, bk[:].rearrange("(c p) o -> p c o", p=128))
    wv_s = load("wv_s", [128, 4, D], BF16, wv[:].rearrange("(c p) d -> p c d", p=128))
    vT_s = load("vT_s", [128, 4, N], BF16, vT[:].rearrange("(c p) n -> p c n", p=128))
    wo_s = load("wo_s", [128, 4, D], BF16, wo[:].rearrange("(c p) d -> p c d", p=128))

    # biases broadcast across partitions once (DMA step-0 partition read)
    bvb = consts.tile([128, D], BF16, name="bvb")
    nc.sync.dma_start(out=bvb, in_=bv[:].to_broadcast((128, D)))
    bob = consts.tile([128, D], F32, name="bob")
    nc.sync.dma_start(out=bob, in_=bo[:].to_broadcast((128, D)))

    KT_s = consts.tile([128, 4, N], BF16)     # K^T, d on partitions
    QT_s = consts.tile([128, 4, NQ], BF16)    # Q^T, d on partitions
    V_s = consts.tile([128, NKT, H, 128], BF16)  # V, k on partitions, [V|1s]
    ctxn_s = consts.tile([128, 4, NQ], BF16)  # normalized ctx^T, d on parts

    nc.vector.memset(V_s[:, :, :, 64:128], 1.0)

    # ---- projections --------------------------------------------------------
    def emit_kproj(dt):
        for kh in range(2):
            st = s_pool.tile([128, 1024], F32, tag="s", name="st_k")
            for kc in range(2):
                for cc in range(4):
                    nc.tensor.matmul(
                        st[:, kc * 512:(kc + 1) * 512],
                        lhsT=wk_s[:, cc, dt * 128:(dt + 1) * 128],
                        rhs=kT_s[:, cc, kh * 1024 + kc * 512: kh * 1024 + (kc + 1) * 512],
                        start=(cc == 0), stop=(cc == 3))
            nc.vector.tensor_scalar_add(
                KT_s[:, dt, kh * 1024:(kh + 1) * 1024], st, bk_s[:, dt, :])

    def emit_qproj(dt):
        st = s_pool.tile([128, 1024], F32, tag="s", name="st_q")
        for qc in range(2):
            for cc in range(4):
                nc.tensor.matmul(
                    st[:, qc * 512:(qc + 1) * 512],
                    lhsT=wq_s[:, cc, dt * 128:(dt + 1) * 128],
                    rhs=qT_s[:, cc, qc * 512:(qc + 1) * 512],
                    start=(cc == 0), stop=(cc == 3))
        nc.vector.tensor_scalar_add(QT_s[:, dt, :], st, bq_s[:, dt, :])

    def emit_vproj(g):  # k tiles 2g, 2g+1
        st = s_pool.tile([128, 1024], F32, tag="s", name="st_v")
        for sub in range(2):
            kt = g * 2 + sub
            sl = st[:, sub * 512:(sub + 1) * 512]
            for cc in range(4):
                nc.tensor.matmul(
                    sl,
                    lhsT=vT_s[:, cc, kt * 128:(kt + 1) * 128],
                    rhs=wv_s[:, cc, :],
                    start=(cc == 0), stop=(cc == 3))
            nc.vector.tensor_add(
                V_s[:, kt, :, 0:64],
                sl.rearrange("p (h w) -> p h w", w=64),
                bvb[:].rearrange("p (h w) -> p h w", w=64))

    # ---- attention: both heads of pair dt, interleaved ----------------------
    def emit_headpair(dt):
        he, ho = 2 * dt, 2 * dt + 1
        ctx_e = c_pool.tile([128, 1024], F32, tag="c", name="ctx_e")
        ctx_o = c_pool.tile([128, 1024], F32, tag="c", name="ctx_o")
        for g in range(8):
            if dt == 0:
                emit_vproj(g)
            if dt < 3 and g == 5:  # prefetch next d-tile's projections
                emit_kproj(dt + 1)
                emit_qproj(dt + 1)
            for sub in range(2):
                kt = g * 2 + sub
                st_e = s_pool.tile([128, 1024], F32, tag="s", name="st_e")
                st_o = s_pool.tile([128, 1024], F32, tag="s", name="st_o")
                # the two heads' S matmuls use PE row groups 0-63 / 64-127:
                # emitted back-to-back they execute concurrently
                for qc in range(2):
                    nc.tensor.matmul(
                        st_e[:, qc * 512:(qc + 1) * 512],
                        lhsT=KT_s[0:64, dt, kt * 128:(kt + 1) * 128],
                        rhs=QT_s[0:64, dt, qc * 512:(qc + 1) * 512],
                        start=True, stop=True)
                    nc.tensor.matmul(
                        st_o[:, qc * 512:(qc + 1) * 512],
                        lhsT=KT_s[64:128, dt, kt * 128:(kt + 1) * 128],
                        rhs=QT_s[64:128, dt, qc * 512:(qc + 1) * 512],
                        start=True, stop=True)
                pt_e = p_pool.tile([128, 1024], BF16, tag="p", name="pt_e")
                if sub == 1:
                    # fast-exp on DVE (affine+int convert) + Pool (bitcast
                    # copy to bf16): frees the Activation engine, which is
                    # otherwise the bottleneck
                    pi = p_pool.tile([128, 1024], I32, tag="pi", name="pi")
                    nc.vector.tensor_scalar(
                        pi, st_e, SCH_A, SCH_B,
                        mybir.AluOpType.mult, mybir.AluOpType.add)
                    nc.gpsimd.tensor_copy(out=pt_e, in_=pi.bitcast(F32))
                else:
                    nc.scalar.activation(pt_e, st_e, Exp, scale=0.125)
                pt_o = p_pool.tile([128, 1024], BF16, tag="p", name="pt_o")
                nc.scalar.activation(pt_o, st_o, Exp, scale=0.125)
                # [V|1s] -> ctx rows 0-63, rowsum replicated on rows 64-127
                for qc in range(2):
                    nc.tensor.matmul(
                        ctx_e[:, qc * 512:(qc + 1) * 512],
                        lhsT=V_s[:, kt, he, :],
                        rhs=pt_e[:, qc * 512:(qc + 1) * 512],
                        start=(kt == 0), stop=(kt == NKT - 1))
                for qc in range(2):
                    nc.tensor.matmul(
                        ctx_o[:, qc * 512:(qc + 1) * 512],
                        lhsT=V_s[:, kt, ho, :],
                        rhs=pt_o[:, qc * 512:(qc + 1) * 512],
                        start=(kt == 0), stop=(kt == NKT - 1))
        # normalize: recip of the replicated rowsum, shift to rows 0-63, mult
        if dbg is not None and dt == 3:
            d_ce = post.tile([128, NQ], F32, tag="dbgc", name="d_ce")
            nc.vector.tensor_copy(out=d_ce, in_=ctx_e)
            nc.sync.dma_start(out=dbg["d_ctxe"][:], in_=d_ce)
            d_co = post.tile([128, NQ], F32, tag="dbgc2", name="d_co")
            nc.vector.tensor_copy(out=d_co, in_=ctx_o)
            nc.sync.dma_start(out=dbg["d_ctxo"][:], in_=d_co)
        # reciprocal_approx_fast is a custom DVE op that misreads PSUM
        # sources: stage the replicated rowsum into SBUF first (ACT copy --
        # the Activation engine has slack, DVE does not)
        rsb_e = post.tile([128, NQ], F32, tag="rsb", name="rsb_e")
        nc.scalar.copy(out=rsb_e[64:128, :], in_=ctx_e[64:128, :])
        rec_e = post.tile([128, NQ], F32, tag="rec", name="rec_e")
        nc.vector.reciprocal_approx_fast(rec_e[64:128, :], rsb_e[64:128, :])
        recs_e = post.tile([128, NQ], F32, tag="recs", name="recs_e")
        nc.sync.dma_start(out=recs_e[0:64, :], in_=rec_e[64:128, :])
        if dbg is not None and dt == 3:
            nc.sync.dma_start(out=dbg["d_rece"][:], in_=rec_e)
        nc.vector.tensor_mul(ctxn_s[0:64, dt, :], ctx_e[0:64, :], recs_e[0:64, :])
        rsb_o = post.tile([128, NQ], F32, tag="rsb", name="rsb_o")
        nc.scalar.copy(out=rsb_o[64:128, :], in_=ctx_o[64:128, :])
        rec_o = post.tile([128, NQ], F32, tag="rec", name="rec_o")
        nc.vector.reciprocal_approx_fast(rec_o[64:128, :], rsb_o[64:128, :])
        recs_o = post.tile([128, NQ], F32, tag="recs", name="recs_o")
        nc.sync.dma_start(out=recs_o[0:64, :], in_=rec_o[64:128, :])
        tmp = post.tile([64, NQ], BF16, tag="tmp", name="tmp")
        nc.vector.tensor_mul(tmp, ctx_o[0:64, :], recs_o[0:64, :])
        # partition shift 0-63 -> 64-127 via SBUF->SBUF DMA
        nc.sync.dma_start(out=ctxn_s[64:128, dt, :], in_=tmp)

    # ---- output projection --------------------------------------------------
    def emit_outproj(g):  # n tiles 2g, 2g+1
        st = c_pool.tile([128, 1024], F32, tag="c", name="st_o2")
        for sub in range(2):
            nt = g * 2 + sub
            sl = st[:, sub * 512:(sub + 1) * 512]
            for dc in range(4):
                nc.tensor.matmul(
                    sl,
                    lhsT=ctxn_s[:, dc, nt * 128:(nt + 1) * 128],
                    rhs=wo_s[:, dc, :],
                    start=(dc == 0), stop=(dc == 3))
            ot = outs.tile([128, D], F32, tag="o", name="ot")
            nc.vector.tensor_add(ot, sl, bob)
            nc.sync.dma_start(out=out[nt * 128:(nt + 1) * 128, :], in_=ot)

    # ---- schedule -----------------------------------------------------------
    emit_kproj(0)
    emit_qproj(0)
    for dt in range(4):
        emit_headpair(dt)
    for g in range(4):
        emit_outproj(g)

    if dbg is not None:
        nc.sync.dma_start(
            out=dbg["d_kt"][:].rearrange("p (c n) -> p c n", c=4), in_=KT_s)
        nc.sync.dma_start(
            out=dbg["d_qt"][:].rearrange("p (c n) -> p c n", c=4), in_=QT_s)
        nc.sync.dma_start(
            out=dbg["d_v"][:].rearrange("p (k h w) -> p k h w", k=NKT, h=H),
            in_=V_s)
        nc.sync.dma_start(
            out=dbg["d_ctxn"][:].rearrange("p (c n) -> p c n", c=4), in_=ctxn_s)


_NC_CACHE = None


def _get_nc():
    global _NC_CACHE
    if _NC_CACHE is None:
        _NC_CACHE = build_nc()
    return _NC_CACHE


def make_in_maps(query, key, value, Wq, bq, Wk, bk, Wv, bv, Wo, bo):
    bf = ml_dtypes.bfloat16
    f = np.float32
    query = np.asarray(query, f)
    key = np.asarray(key, f)
    value = np.asarray(value, f)
    shared = {
        "wq": np.asarray(Wq, f).astype(bf),
        "wk": np.asarray(Wk, f).astype(bf),
        "wv": np.asarray(Wv, f).astype(bf),
        "wo": np.asarray(Wo, f).astype(bf),
        "bq": np.asarray(bq, f).reshape(D, 1),
        "bk": np.asarray(bk, f).reshape(D, 1),
        "bv": np.asarray(bv, f).astype(bf).reshape(1, D),
        "bo": np.asarray(bo, f).reshape(1, D),
    }
    kTs = [np.ascontiguousarray(key[b].T).astype(bf) for b in range(B)]
    vTs = [np.ascontiguousarray(value[b].T).astype(bf) for b in range(B)]
    in_maps = []
    for c in range(8):
        b, half = c // 2, c % 2
        m = dict(shared)
        m["qT"] = np.ascontiguousarray(
            query[b, half * NQ:(half + 1) * NQ, :].T).astype(bf)
        m["kT"] = kTs[b]
        m["vT"] = vTs[b]
        in_maps.append(m)
    return in_maps


def run(inputs, trace=False):
    nc = _get_nc()
    in_maps = make_in_maps(**inputs)
    res = run_bass_kernel_spmd(nc, in_maps, core_ids=list(range(8)), trace=trace)
    out = np.empty((B, N, D), np.float32)
    for c in range(8):
        b, half = c // 2, c % 2
        out[b, half * NQ:(half + 1) * NQ, :] = res.results[c]["out"]
    return out, res


def kernel(**inputs):
    out, _ = run(inputs, trace=False)
    return out
